# revision 10
# baseline (speedup 1.0000x reference)
"""Trainium2 Bass kernel for nn_DecoderRoPEBlock (B=4, LQ=1024, LC=512,
E=1024, H=16, FF=4096) running SPMD on 8 NeuronCores.

Sharding: 8 cores = (batch, striped query-tiles); zero collectives.
Stage-1 causal self-attention K/V are recomputed per core from the
original x (causality means the pre-residual x suffices), so each core
produces its 512 output rows independently. Causal striping: each core
owns interleaved 128-token query tiles, so score/exp/AV work shrinks by
the causal factor uniformly across all cores.

v2 optimizations over the baseline:
- Score tiles hold a pair of key-tiles with equal causal offset, so one
  Exp activation covers two key tiles (halves ACT instruction count).
- Softmax denominator reciprocal on DVE (vector.reciprocal) instead of
  Ln+Exp on the Scalar engine; the recip broadcast is read from PSUM by
  the normalizing STT (no SBUF staging copy).
- Cross-attention K/V projections (which depend only on `context`) are
  emitted as PE filler work interleaved into the stage-1 attention loop,
  covering the Tensor-engine bubbles left by the softmax critical path.
- PSUM plan: mm ring (2 banks) + score-pair ring (2x2 banks) + o_pair
  (2 banks) = 8 banks; fc2 reuses all four as 8 accumulators.
- Mask loads shrunk to the single 128-col block each key tile needs.
"""
import sys
sys.path.insert(0, '/opt/trn_rl_repo')
from contextlib import ExitStack

import numpy as np
import ml_dtypes

import concourse.bass as bass
from concourse import bacc
import concourse.tile as tile
import concourse.mybir as mybir

f32 = mybir.dt.float32
bf16 = mybir.dt.bfloat16
f8 = mybir.dt.float8e4
AF = mybir.ActivationFunctionType
ALU = mybir.AluOpType
EPS = 1e-6
P = 128


class Cfg:
    def __init__(self, E, H, LQ, LC, B, FF, n_cores):
        self.E, self.H, self.LQ, self.LC, self.B, self.FF = E, H, LQ, LC, B, FF
        self.HD = E // H
        assert self.HD == 64, "rope layout assumes head dim 64"
        self.n_cores = n_cores
        self.qsplit = n_cores // B
        assert B * self.qsplit == n_cores
        self.Lq = LQ // self.qsplit
        assert self.Lq <= 512
        self.Lk = LQ
        self.Lc = LC
        self.nec = E // P
        self.nkt = self.Lk // P
        self.nct = self.Lc // P
        self.npr = H // 2
        assert self.npr == self.nec, "2 heads per 128-chunk layout"
        self.nft = FF // P
        self.NT = 512


def su_list_sa(c):
    return [min(P * (kt % (c.nkt // 2)), c.Lq - P) for kt in range(c.nkt)]


def _swap32_dma(nc, dst, src, L):
    """dst = src with 32-blocks swapped inside each 64-block (partitions).
    Issued on the gpsimd queue to keep the sync HWDGE queue free for
    weight streaming."""
    nc.gpsimd.dma_start(out=dst[0:32], in_=src[32:64])
    nc.gpsimd.dma_start(out=dst[32:64], in_=src[0:32])
    nc.gpsimd.dma_start(out=dst[64:96], in_=src[96:128])
    nc.gpsimd.dma_start(out=dst[96:128], in_=src[64:96])


def _steer_act_tables(arch):
    """Keep Ln/Exp/Square together in natural_log_exp_and_others so the
    LN rstd pipeline (Ln -> Exp) and softmax Exp never reload tables."""
    from concourse.hw_specs import get_activation_tables
    try:
        tabs = get_activation_tables(arch)
    except Exception:
        return
    target = 'natural_log_exp_and_others'
    if target not in tabs:
        return
    keep = tabs[target]
    for name, s in tabs.items():
        if name == target:
            continue
        if AF.Exp in keep:
            s.discard(AF.Exp)
        if AF.Ln in keep:
            s.discard(AF.Ln)
        if AF.Square in keep:
            s.discard(AF.Square)


def build_core_program(cfg: Cfg):
    c = cfg
    nc = bacc.Bacc()
    _steer_act_tables(nc.m.arch)

    d_xT = nc.declare_dram_parameter("xT", [c.E, c.Lq], f32, isOutput=False)
    d_xT16 = nc.declare_dram_parameter("xT16", [c.E, c.Lk], bf16, isOutput=False)
    d_ctxT16 = nc.declare_dram_parameter("ctxT16", [c.E, c.Lc], bf16, isOutput=False)
    d_mask = nc.declare_dram_parameter("mask16", [c.Lk, P], bf16, isOutput=False)
    WNAMES = ["sa_q", "sa_k", "sa_v", "sa_p", "ca_q", "ca_k", "ca_v", "ca_p"]
    d_w = {n: nc.declare_dram_parameter("w_" + n, [c.E, c.E], bf16, isOutput=False)
           for n in WNAMES}
    d_fc1 = nc.declare_dram_parameter("w_fc1", [c.E, c.FF], bf16, isOutput=False)
    d_fc2 = nc.declare_dram_parameter("w_fc2", [c.FF, c.E], f8, isOutput=False)
    d_cq = nc.declare_dram_parameter("cos_q", [P, c.Lq], bf16, isOutput=False)
    d_sq = nc.declare_dram_parameter("sin_q", [P, c.Lq], bf16, isOutput=False)
    d_ck = nc.declare_dram_parameter("cos_k", [P, c.Lk], bf16, isOutput=False)
    d_sk = nc.declare_dram_parameter("sin_k", [P, c.Lk], bf16, isOutput=False)
    d_cc = nc.declare_dram_parameter("cos_c", [P, c.Lc], bf16, isOutput=False)
    d_sc = nc.declare_dram_parameter("sin_c", [P, c.Lc], bf16, isOutput=False)
    d_g = nc.declare_dram_parameter("gvec", [c.E, 3], f32, isOutput=False)
    d_out = nc.declare_dram_parameter("outT", [c.E, c.Lq], f32, isOutput=True)

    Lq, Lk, Lc, nec, nkt, nct, npr, nft = (
        c.Lq, c.Lk, c.Lc, c.nec, c.nkt, c.nct, c.npr, c.nft)
    VNT = min(c.NT, c.E)
    n_vnt = c.E // VNT
    KNT = min(c.NT, Lk)
    n_knt = Lk // KNT
    NG = 4 if nft % 4 == 0 else 1          # fc1/fc2 stream groups
    FG = c.FF // NG                        # cols per fc1 group
    nftg = nft // NG

    with tile.TileContext(nc) as tc, ExitStack() as ctx:
        # -------------------- pools --------------------
        p_x = ctx.enter_context(tc.tile_pool(name="p_x", bufs=1))
        p_h = ctx.enter_context(tc.tile_pool(name="p_h", bufs=1))
        p_big = ctx.enter_context(tc.tile_pool(name="p_big", bufs=9))
        p_s1k = ctx.enter_context(tc.tile_pool(name="p_s1k", bufs=32))
        p_ex = ctx.enter_context(tc.tile_pool(name="p_ex", bufs=3))
        p_w = ctx.enter_context(tc.tile_pool(name="p_w", bufs=8))
        p_wca = ctx.enter_context(tc.tile_pool(name="p_wca", bufs=8))
        p_ca = ctx.enter_context(tc.tile_pool(name="p_ca", bufs=1))
        p_per = ctx.enter_context(tc.tile_pool(name="p_per", bufs=1))
        p_scr = ctx.enter_context(tc.tile_pool(name="p_scr", bufs=1))
        p_sm = ctx.enter_context(tc.tile_pool(name="p_sm", bufs=1))
        p_t = ctx.enter_context(tc.tile_pool(name="p_t", bufs=3))
        ps_mm = ctx.enter_context(tc.tile_pool(name="ps_mm", bufs=2, space="PSUM"))
        ps_sc = ctx.enter_context(tc.tile_pool(name="ps_sc", bufs=2, space="PSUM"))
        ps_o = ctx.enter_context(tc.tile_pool(name="ps_o", bufs=1, space="PSUM"))

        # -------------------- prologue loads --------------------
        x16 = []
        for e in range(nec):
            t = p_h.tile([P, Lk], bf16, tag=f"h{e}", name=f"h{e}")
            nc.sync.dma_start(out=t[:], in_=d_xT16[e * P:(e + 1) * P, :])
            x16.append(t)
        cq = p_per.tile([P, Lq], bf16, tag="cq", name="cq")
        sq = p_per.tile([P, Lq], bf16, tag="sq", name="sq")
        ck = p_per.tile([P, Lk], bf16, tag="ck", name="ck")
        sk = p_per.tile([P, Lk], bf16, tag="sk", name="sk")
        ccos = p_per.tile([P, Lc], bf16, tag="ccos", name="ccos")
        csin = p_per.tile([P, Lc], bf16, tag="csin", name="csin")
        for t, d in ((cq, d_cq), (sq, d_sq), (ck, d_ck), (sk, d_sk),
                     (ccos, d_cc), (csin, d_sc)):
            nc.sync.dma_start(out=t[:], in_=d[:, :])
        masks = []
        for kt in range(nkt):
            t = p_per.tile([P, P], bf16, tag=f"mask{kt}", name=f"mask{kt}")
            nc.gpsimd.dma_start(out=t[:], in_=d_mask[kt * P:(kt + 1) * P, :])
            masks.append(t)
        ctx16 = []
        for e in range(nec):
            t = p_ca.tile([P, Lc], bf16, tag=f"ctx{e}", name=f"ctx{e}")
            nc.gpsimd.dma_start(out=t[:], in_=d_ctxT16[e * P:(e + 1) * P, :])
            ctx16.append(t)
        gsb = p_per.tile([P, nec, 3], f32, tag="g", name="g")
        for e in range(nec):
            nc.gpsimd.dma_start(out=gsb[:, e, :], in_=d_g[e * P:(e + 1) * P, :])
        ones_col = p_per.tile([P, 1], bf16, tag="ones_col", name="ones_col")
        nc.vector.memset(ones_col[:], 1.0)
        ones_row = p_per.tile([1, P], bf16, tag="ones_row", name="ones_row")
        nc.vector.memset(ones_row[:], 1.0)
        epsb = p_per.tile([1, 1], f32, tag="epsb", name="epsb")
        nc.vector.memset(epsb[:], EPS)

        def load_w(dram, pool=None, tag="wproj"):
            pool = pool or p_w
            tiles = []
            for e in range(nec):
                t = pool.tile([P, c.E], bf16, tag=tag, name=f"{tag}_{e}")
                nc.sync.dma_start(out=t[:], in_=dram[e * P:(e + 1) * P, :])
                tiles.append(t)
            return tiles

        # ca_k / ca_v weights up front; their GEMMs run inside stage 1
        w_cak = load_w(d_w["ca_k"], pool=p_wca, tag="wca")
        w_cav = load_w(d_w["ca_v"], pool=p_wca, tag="wca")

        # xT (f32 residual base) is only needed from the sa_p residual on
        xT = []
        for e in range(nec):
            t = p_x.tile([P, Lq], f32, tag=f"x{e}", name=f"x{e}")
            nc.sync.dma_start(out=t[:], in_=d_xT[e * P:(e + 1) * P, :])
            xT.append(t)

        # ==================== LN ====================
        def layer_norm(src_tiles, L, src_f32=None):
            """LN over E of transposed src [E, L] -> bf16 h tiles (p_h h{e})."""
            n_lt = max(1, L // 512)
            LT = L // n_lt
            # phase A: per-L-subtile stats -> rstd, cc vectors (SBUF)
            rstds, ccvs = [], []
            for lt in range(n_lt):
                sl = slice(lt * LT, (lt + 1) * LT)
                sq_t = []
                for e in range(nec):
                    s = p_scr.tile([P, LT], bf16, tag=f"sq{e}", name=f"sq{e}")
                    nc.vector.tensor_mul(s[:], src_tiles[e][:, sl],
                                         src_tiles[e][:, sl])
                    sq_t.append(s)
                s1 = ps_mm.tile([1, LT], f32, tag="mm", name="s1")
                s2 = ps_mm.tile([1, LT], f32, tag="mm", name="s2")
                for e in range(nec):
                    nc.tensor.matmul(s1[:], ones_col[:], src_tiles[e][:, sl],
                                     start=(e == 0), stop=(e == nec - 1))
                for e in range(nec):
                    nc.tensor.matmul(s2[:], ones_col[:], sq_t[e][:],
                                     start=(e == 0), stop=(e == nec - 1))
                mu = p_sm.tile([1, LT], f32, tag="lnsc", name="mu", bufs=4)
                nc.scalar.mul(mu[:], s1[:], 1.0 / c.E)
                mu2 = p_sm.tile([1, LT], f32, tag="lnsc", name="mu2", bufs=4)
                nc.scalar.square(mu2[:], mu[:])
                s2c = p_sm.tile([1, LT], f32, tag="lnsc", name="s2c", bufs=4)
                nc.scalar.mul(s2c[:], s2[:], 1.0 / c.E)
                var = p_sm.tile([1, LT], f32, tag="lnsc", name="var", bufs=4)
                nc.vector.tensor_sub(var[:], s2c[:], mu2[:])
                lnv = p_sm.tile([1, LT], f32, tag="lnsc", name="lnv", bufs=4)
                nc.scalar.activation(out=lnv[:], in_=var[:], func=AF.Ln,
                                     bias=epsb[:])
                rstd = p_sm.tile([1, LT], bf16, tag="rstd", name="rstd", bufs=2)
                nc.scalar.activation(out=rstd[:], in_=lnv[:], func=AF.Exp,
                                     scale=-0.5)
                ccv = p_sm.tile([1, LT], bf16, tag="ccv", name="ccv", bufs=2)
                nc.vector.tensor_mul(ccv[:], mu[:], rstd[:])
                rstds.append(rstd)
                ccvs.append(ccv)
            # phase B: broadcasts (lt=0 -> mm ring, lt=1 -> sc ring)
            rstd_bs, cc_bs = [], []
            for lt in range(n_lt):
                if lt == 0:
                    rb = ps_mm.tile([P, LT], f32, tag="mm", name="rb")
                    cb = ps_mm.tile([P, LT], f32, tag="mm", name="cb")
                else:
                    rb = ps_sc.tile([P, LT], f32, tag="sc", name="rb1")
                    cb = ps_sc.tile([P, LT], f32, tag="sc", name="cb1")
                nc.tensor.matmul(rb[:], ones_row[:], rstds[lt][:],
                                 start=True, stop=True)
                nc.tensor.matmul(cb[:], ones_row[:], ccvs[lt][:],
                                 start=True, stop=True)
                rstd_bs.append(rb)
                cc_bs.append(cb)
            # phase C: apply; fully read src chunk e before writing h[e]
            hs = [p_h.tile([P, L], bf16, tag=f"h{e}", name=f"hln{e}")
                  for e in range(nec)]
            for e in range(nec):
                src = src_f32[e] if src_f32 is not None else src_tiles[e]
                tmps = []
                for lt in range(n_lt):
                    sl = slice(lt * LT, (lt + 1) * LT)
                    tmp = p_t.tile([P, LT], bf16, tag="lntmp", name="lntmp", bufs=2)
                    nc.vector.tensor_mul(tmp[:], src[:, sl], rstd_bs[lt][:])
                    tmps.append(tmp)
                for lt in range(n_lt):
                    sl = slice(lt * LT, (lt + 1) * LT)
                    nc.vector.tensor_sub(hs[e][:, sl], tmps[lt][:], cc_bs[lt][:])
            return hs

        # ==================== projections ====================
        def project_T(w_tiles, rhs_tiles, L, out_tag, pool):
            """[E_out, L] = sum_e w[e].T @ rhs[e]; returns nec bf16 tiles."""
            outs = []
            for eo in range(nec):
                ps = ps_mm.tile([P, L], f32, tag="mm", name="mm")
                for ei in range(nec):
                    nc.tensor.matmul(ps[:], w_tiles[ei][:, eo * P:(eo + 1) * P],
                                     rhs_tiles[ei][:], start=(ei == 0),
                                     stop=(ei == nec - 1))
                o = pool.tile([P, L], bf16, tag=f"{out_tag}{eo}", name=f"{out_tag}{eo}")
                nc.scalar.copy(o[:], ps[:])
                outs.append(o)
            return outs

        def rope_combine(dst_ap, raw_tile, cos_ap, sin_ap, L):
            swp = p_t.tile([P, L], bf16, tag="ropeswp", name="ropeswp", bufs=2)
            _swap32_dma(nc, swp[:], raw_tile[:], L)
            t1 = p_t.tile([P, L], bf16, tag="ropet1", name="ropet1", bufs=2)
            nc.vector.tensor_mul(t1[:], raw_tile[:], cos_ap)
            t2 = p_t.tile([P, L], bf16, tag="ropet2", name="ropet2", bufs=2)
            nc.vector.tensor_mul(t2[:], swp[:], sin_ap)
            nc.vector.tensor_add(dst_ap, t1[:], t2[:])

        def v_project2(w_tiles, rhs_tiles, n_kt, v_tag, use_vector):
            v_sb = []
            for kt in range(n_kt):
                vt = p_per.tile([P, c.H, 65], bf16, tag=f"{v_tag}{kt}",
                                name=f"{v_tag}{kt}")
                nc.vector.memset(vt[:, :, 64:65], 1.0)
                v_sb.append(vt)
            for kt in range(n_kt):
                for vn in range(n_vnt):
                    ps = ps_mm.tile([P, VNT], f32, tag="mm", name="mm")
                    for ei in range(nec):
                        nc.tensor.matmul(
                            ps[:],
                            rhs_tiles[ei][:, kt * P:(kt + 1) * P],
                            w_tiles[ei][:, vn * VNT:(vn + 1) * VNT],
                            start=(ei == 0), stop=(ei == nec - 1))
                    nh = VNT // 64
                    dst = v_sb[kt][:, vn * nh:(vn + 1) * nh, 0:64]
                    src = ps[:].rearrange("p (nh d) -> p nh d", d=64)
                    if use_vector:
                        nc.vector.tensor_copy(dst, src)
                    else:
                        nc.scalar.copy(dst, src)
            return v_sb

        # ==================== attention ====================
        def attention(q_tiles, k_tiles, v_sb, n_kt, use_mask, su_list,
                      fillers=None, fill_per_pr=0):
            """Key tiles are processed in pairs (kt, kt + n_kt//2) with equal
            causal start column su, so one Exp covers both. Returns Onorm
            tiles (tag qt{pr}). fillers: iterator of thunks emitted between
            pr iterations to keep the PE busy during softmax latency."""
            npair = n_kt // 2
            on_tiles = []
            for pr in range(npr):
                qch = q_tiles[pr]
                kch = k_tiles[pr]
                o_pair = ps_o.tile([65, 2, Lq], f32, tag="o", name="o_pair")
                for hh_i, (hh, pbase) in enumerate(
                        ((2 * pr, 0), (2 * pr + 1, 64))):
                    exs = []
                    for pi in range(npair):
                        kts = (pi, pi + npair)
                        su = su_list[kts[0]]
                        assert su == su_list[kts[1]]
                        s_ps = ps_sc.tile([P, 2, Lq], f32, tag="sc", name="s_ps")
                        for i, kt in enumerate(kts):
                            nc.tensor.matmul(
                                s_ps[:, i, su:],
                                kch[pbase:pbase + 64, kt * P:(kt + 1) * P],
                                qch[pbase:pbase + 64, su:],
                                start=True, stop=True)
                        ex = p_ex.tile([P, 2, Lq], bf16, tag="ex", name="ex")
                        nc.scalar.activation(out=ex[:, :, su:],
                                             in_=s_ps[:, :, su:],
                                             func=AF.Exp, scale=0.125)
                        if use_mask:
                            for i, kt in enumerate(kts):
                                nc.vector.tensor_mul(
                                    ex[:, i, su:su + P],
                                    ex[:, i, su:su + P],
                                    masks[kt][:, :])
                        exs.append((ex, su, kts))
                    # AV chain over all key tiles for this head
                    nmm = 0
                    for (ex, su, kts) in exs:
                        for i, kt in enumerate(kts):
                            nc.tensor.matmul(
                                o_pair[:, hh_i, su:],
                                v_sb[kt][:, hh, :],
                                ex[:, i, su:],
                                start=(nmm == 0), stop=(nmm == n_kt - 1))
                            nmm += 1
                # normalization: recip on DVE, broadcast via PE, STT on DVE
                rec = p_sm.tile([1, 2, Lq], bf16, tag="rec", name="rec", bufs=1)
                with nc.allow_low_precision(reason="softmax denom recip"):
                    nc.vector.reciprocal(rec[:], o_pair[64:65, :, :])
                db_ps = ps_mm.tile([P, Lq], f32, tag="mm", name="db_ps")
                for h in (0, 1):
                    nc.tensor.matmul(db_ps[h * 64:(h + 1) * 64, :],
                                     ones_row[:, 0:64], rec[:, h, :],
                                     start=True, stop=True)
                db = p_t.tile([P, Lq], bf16, tag="db", name="db", bufs=2)
                nc.vector.tensor_copy(db[:], db_ps[:])
                on = p_per.tile([P, Lq], bf16, tag=f"qt{pr}", name=f"on{pr}")
                for h, pbase in ((0, 0), (1, 64)):
                    nc.vector.scalar_tensor_tensor(
                        out=on[pbase:pbase + 64, :],
                        in0=o_pair[0:64, h, :], scalar=1.0,
                        in1=db[h * 64:(h + 1) * 64, :],
                        op0=ALU.bypass, op1=ALU.mult)
                on_tiles.append(on)
                if fillers is not None:
                    for _ in range(fill_per_pr):
                        try:
                            next(fillers)()
                        except StopIteration:
                            break
            return on_tiles

        def proj_residual(w_tiles, src_tiles, g_idx):
            for e in range(nec):
                ps = ps_mm.tile([P, Lq], f32, tag="mm", name="mm")
                for ei in range(nec):
                    nc.tensor.matmul(ps[:], w_tiles[ei][:, e * P:(e + 1) * P],
                                     src_tiles[ei][:], start=(ei == 0),
                                     stop=(ei == nec - 1))
                nc.vector.scalar_tensor_tensor(
                    out=xT[e][:], in0=ps[:], scalar=gsb[:, e, g_idx:g_idx + 1],
                    in1=xT[e][:], op0=ALU.mult, op1=ALU.add)

        # ========== cross-attn K/V filler work (runs inside stage 1) =====
        kt2 = [None] * nec
        v2 = []
        for kt in range(nct):
            vt = p_ca.tile([P, c.H, 65], bf16, tag=f"v2_{kt}", name=f"v2_{kt}")
            nc.vector.memset(vt[:, :, 64:65], 1.0)
            v2.append(vt)

        def ca_k_chunk(eo):
            def thunk():
                ps = ps_mm.tile([P, Lc], f32, tag="mm", name="cak")
                for ei in range(nec):
                    nc.tensor.matmul(ps[:],
                                     w_cak[ei][:, eo * P:(eo + 1) * P],
                                     ctx16[ei][:], start=(ei == 0),
                                     stop=(ei == nec - 1))
                raw = p_t.tile([P, Lc], bf16, tag="k2raw", name="k2raw", bufs=2)
                nc.vector.tensor_copy(raw[:], ps[:])
                full = p_ca.tile([P, Lc], bf16, tag=f"kt2_{eo}", name=f"kt2_{eo}")
                rope_combine(full[:], raw, ccos[:], csin[:], Lc)
                kt2[eo] = full
            return thunk

        def ca_v_chunk(kt, vn):
            def thunk():
                ps = ps_mm.tile([P, VNT], f32, tag="mm", name="cav")
                for ei in range(nec):
                    nc.tensor.matmul(
                        ps[:],
                        ctx16[ei][:, kt * P:(kt + 1) * P],
                        w_cav[ei][:, vn * VNT:(vn + 1) * VNT],
                        start=(ei == 0), stop=(ei == nec - 1))
                nh = VNT // 64
                nc.vector.tensor_copy(
                    v2[kt][:, vn * nh:(vn + 1) * nh, 0:64],
                    ps[:].rearrange("p (nh d) -> p nh d", d=64))
            return thunk

        ca_fillers = iter(
            [ca_k_chunk(eo) for eo in range(nec)] +
            [ca_v_chunk(kt, vn) for kt in range(nct) for vn in range(n_vnt)])

        # ==================== STAGE 1: causal self-attention ============
        h1 = layer_norm(x16, Lk)
        hq = [t[:, 0:Lq] for t in h1]

        w = load_w(d_w["sa_q"])
        q_raw = project_T(w, hq, Lq, "scrB", p_scr)
        qt1 = []
        for pr in range(npr):
            q = p_per.tile([P, Lq], bf16, tag=f"qt{pr}", name=f"qt{pr}")
            rope_combine(q[:], q_raw[pr], cq[:], sq[:], Lq)
            qt1.append(q)
        w = load_w(d_w["sa_k"])
        kt1 = []
        k_raw_nt = []
        for nt in range(n_knt):
            sl = slice(nt * KNT, (nt + 1) * KNT)
            tag = "sq" if nt == 0 else "scrB"
            kr = project_T(w, [t[:, sl] for t in h1], KNT, tag, p_scr)
            k_raw_nt.append(kr)
        for e in range(nec):
            full = p_big.tile([P, Lk], bf16, tag="big", name="big")
            for nt in range(n_knt):
                sl = slice(nt * KNT, (nt + 1) * KNT)
                rope_combine(full[:, sl], k_raw_nt[nt][e],
                             ck[:, sl], sk[:, sl], KNT)
            kt1.append(full)
        w = load_w(d_w["sa_v"])
        v1 = v_project2(w, h1, nkt, "v", use_vector=False)
        su_sa = su_list_sa(c)
        on1 = attention(qt1, kt1, v1, nkt, True, su_sa,
                        fillers=ca_fillers, fill_per_pr=2)
        # drain any remaining filler work
        for th in ca_fillers:
            th()
        w = load_w(d_w["sa_p"])
        proj_residual(w, on1, 0)

        # ==================== STAGE 2: cross-attention ==================
        x16_2 = []
        for e in range(nec):
            t = p_scr.tile([P, Lq], bf16, tag=f"scrB{e}", name=f"scrB{e}")
            nc.scalar.copy(t[:], xT[e][:])
            x16_2.append(t)
        h2 = layer_norm(x16_2, Lq, src_f32=xT)
        w = load_w(d_w["ca_q"])
        q_raw = project_T(w, h2, Lq, "scrB", p_scr)
        qt2 = []
        for pr in range(npr):
            q = p_per.tile([P, Lq], bf16, tag=f"qt{pr}", name=f"qt{pr}")
            rope_combine(q[:], q_raw[pr], cq[:], sq[:], Lq)
            qt2.append(q)
        # prefetch fc1 group 0 weights during stage-2 attention
        wf0 = []
        for e in range(nec):
            t = p_big.tile([P, FG], bf16, tag="big", name="big")
            nc.sync.dma_start(out=t[:], in_=d_fc1[e * P:(e + 1) * P, 0:FG])
            wf0.append(t)
        on2 = attention(qt2, kt2, v2, nct, False, [0] * nct)
        w = load_w(d_w["ca_p"])
        proj_residual(w, on2, 1)

        # ==================== STAGE 3: MLP ==============================
        x16_3 = []
        for e in range(nec):
            t = p_scr.tile([P, Lq], bf16, tag=f"scrB{e}", name=f"scrB{e}")
            nc.scalar.copy(t[:], xT[e][:])
            x16_3.append(t)
        h3 = layer_norm(x16_3, Lq, src_f32=xT)
        a_tiles = []
        for grp in range(NG):
            if grp == 0:
                wf = wf0
            else:
                wf = []
                for e in range(nec):
                    t = p_big.tile([P, FG], bf16, tag="big", name="big")
                    nc.sync.dma_start(
                        out=t[:], in_=d_fc1[e * P:(e + 1) * P,
                                            grp * FG:(grp + 1) * FG])
                    wf.append(t)
            for ft in range(nftg):
                ps = ps_mm.tile([P, Lq], f32, tag="mm", name="mm")
                for ei in range(nec):
                    nc.tensor.matmul(ps[:], wf[ei][:, ft * P:(ft + 1) * P],
                                     h3[ei][:], start=(ei == 0),
                                     stop=(ei == nec - 1))
                a = p_s1k.tile([P, Lq], f8, tag="s1k", name="s1k")
                nc.scalar.activation(out=a[:], in_=ps[:],
                                     func=AF.Gelu_apprx_tanh)
                a_tiles.append(a)
        # fc2: all 8 psum banks as accumulators, stream fc2 weights
        accs = []
        for i in range(2):
            big = ps_sc.tile([P, 2, Lq], f32, tag="sc", name=f"fc2sc{i}")
            accs.append(big[:, 0, :])
            accs.append(big[:, 1, :])
        op = ps_o.tile([P, 2, Lq], f32, tag="o", name="fc2o")
        accs.append(op[:, 0, :])
        accs.append(op[:, 1, :])
        for i in range(2):
            accs.append(ps_mm.tile([P, Lq], f32, tag="mm", name=f"fc2mm{i}"))
        assert len(accs) >= nec
        for fi in range(nft):
            wt = p_w.tile([P, c.E], f8, tag="wproj", name="wproj")
            nc.sync.dma_start(out=wt[:], in_=d_fc2[fi * P:(fi + 1) * P, :])
            for e in range(nec):
                nc.tensor.matmul(accs[e][:], wt[:, e * P:(e + 1) * P],
                                 a_tiles[fi][:], start=(fi == 0),
                                 stop=(fi == nft - 1))
        for e in range(nec):
            nc.vector.scalar_tensor_tensor(
                out=xT[e][:], in0=accs[e][:], scalar=gsb[:, e, 2:3],
                in1=xT[e][:], op0=ALU.mult, op1=ALU.add)

        # ==================== output ====================
        for e in range(nec):
            nc.sync.dma_start(out=d_out[e * P:(e + 1) * P, :], in_=xT[e][:])

    nc.finalize()
    return nc


# ======================================================================
# Host-side preparation
# ======================================================================
def rope_tables(positions, HD, dtype=np.float32):
    inv_freq = 1.0 / (10000.0 ** (np.arange(0, HD, 2, dtype=np.float64) / HD))
    ang = positions[None, :].astype(np.float64) * inv_freq[:, None]
    cos, sin = np.cos(ang), np.sin(ang)
    c64 = np.concatenate([cos, cos], 0)
    s64 = np.concatenate([-sin, sin], 0)
    return (np.concatenate([c64, c64], 0).astype(dtype),
            np.concatenate([s64, s64], 0).astype(dtype))


def rope_perm(E, HD):
    H = E // HD
    perm = np.zeros(E, dtype=np.int64)
    for h in range(H):
        base = h * HD
        perm[base:base + 32] = base + np.arange(0, HD, 2)
        perm[base + 32:base + HD] = base + np.arange(1, HD, 2)
    return perm


def to_bf(a):
    return np.asarray(a, dtype=np.float32).astype(ml_dtypes.bfloat16)


def host_prep(inputs, cfg: Cfg):
    c = cfg
    E, HD = c.E, c.HD
    perm = rope_perm(E, HD)

    def ln_fold(w, nw, do_perm):
        weff = np.asarray(w, np.float64)
        if nw is not None:
            weff = weff * np.asarray(nw, np.float64)[None, :]
        if do_perm:
            weff = weff[perm, :]
        return weff.T

    x = np.asarray(inputs['x'], np.float32)
    ctxv = np.asarray(inputs['context'], np.float32)
    am = np.asarray(inputs['attn_mask'])
    n1w, n2w, n3w = (np.asarray(inputs[k], np.float32).reshape(-1)
                     for k in ('n1_w', 'n2_w', 'n3_w'))
    for nb in ('n1_b', 'n2_b', 'n3_b', 'sa_qb', 'sa_kb', 'sa_vb', 'sa_pb',
               'ca_qb', 'ca_kb', 'ca_vb', 'ca_pb', 'fc1_b', 'fc2_b'):
        assert not np.any(np.asarray(inputs[nb])), f"nonzero bias {nb}"

    shared = {
        'w_sa_q': to_bf(ln_fold(inputs['sa_qw'], n1w, True)),
        'w_sa_k': to_bf(ln_fold(inputs['sa_kw'], n1w, True)),
        'w_sa_v': to_bf(ln_fold(inputs['sa_vw'], n1w, False)),
        'w_sa_p': to_bf(np.asarray(inputs['sa_pw'], np.float64).T),
        'w_ca_q': to_bf(ln_fold(inputs['ca_qw'], n2w, True)),
        'w_ca_k': to_bf(ln_fold(inputs['ca_kw'], None, True)),
        'w_ca_v': to_bf(np.asarray(inputs['ca_vw'], np.float64).T),
        'w_ca_p': to_bf(np.asarray(inputs['ca_pw'], np.float64).T),
        'w_fc1': to_bf(ln_fold(inputs['fc1_w'], n3w, False)),
        'w_fc2': np.ascontiguousarray(
            (np.asarray(inputs['fc2_w'], np.float64).T * 32.0)
            .astype(np.float32).astype(ml_dtypes.float8_e4m3fn)),
        'gvec': np.ascontiguousarray(np.stack(
            [np.asarray(inputs['g_msa'], np.float32).reshape(-1),
             np.asarray(inputs['g_ca'], np.float32).reshape(-1),
             np.asarray(inputs['g_mlp'], np.float32).reshape(-1) / 32.0], 1)),
    }
    mask_T = (~am).astype(np.float32).T       # [k, q] multiplicative
    cc_np, sc_np = rope_tables(np.arange(c.Lc), HD)
    shared['cos_c'] = to_bf(cc_np)
    shared['sin_c'] = to_bf(sc_np)
    su_sa = su_list_sa(c)

    in_maps = []
    for core in range(c.n_cores):
        b = core // c.qsplit
        qh = core % c.qsplit
        perm_tok = _core_token_perm(c, qh)
        own = perm_tok[:c.Lq]
        cq_np, sq_np = rope_tables(own.astype(np.float64), HD)
        ck_np, sk_np = rope_tables(perm_tok.astype(np.float64), HD)
        xb_T = x[b].T                          # [E, Lk]
        # per-kt [P, P] mask blocks at the causal start column su
        mask_full = to_bf(mask_T[perm_tok][:, own])     # [Lk, Lq]
        mask_blk = np.zeros((c.Lk, P), ml_dtypes.bfloat16)
        for kt in range(c.nkt):
            su = su_sa[kt]
            mask_blk[kt * P:(kt + 1) * P, :] = \
                mask_full[kt * P:(kt + 1) * P, su:su + P]
        m = dict(shared)
        m.update({
            'xT': np.ascontiguousarray(xb_T[:, own]),
            'xT16': np.ascontiguousarray(to_bf(xb_T[:, perm_tok])),
            'ctxT16': np.ascontiguousarray(to_bf(ctxv[b].T)),
            'mask16': np.ascontiguousarray(mask_blk),
            'cos_q': to_bf(cq_np), 'sin_q': to_bf(sq_np),
            'cos_k': to_bf(ck_np), 'sin_k': to_bf(sk_np),
        })
        in_maps.append(m)
    return in_maps


def _core_token_perm(c, qh):
    """Own (striped) 128-token tiles first, then the other range's tiles."""
    ntile = c.Lk // 128
    if c.qsplit == 1:
        order = list(range(ntile))
    else:
        own_t = list(range(qh, ntile, c.qsplit))
        oth_t = [t for t in range(ntile) if t not in own_t]
        order = own_t + oth_t
    return np.concatenate(
        [np.arange(t * 128, (t + 1) * 128) for t in order])


def assemble_output(results, cfg: Cfg):
    c = cfg
    out = np.zeros((c.B, c.LQ, c.E), np.float32)
    for core in range(c.n_cores):
        b = core // c.qsplit
        qh = core % c.qsplit
        own = _core_token_perm(c, qh)[:c.Lq]
        out[b, own, :] = np.asarray(results[core]['outT']).T
    return out


_CFG = Cfg(E=1024, H=16, LQ=1024, LC=512, B=4, FF=4096, n_cores=8)
_CACHE = {}


def kernel(**inputs):
    from concourse.bass_utils import run_bass_kernel_spmd
    cfg = _CFG
    in_maps = host_prep(inputs, cfg)
    if 'nc' not in _CACHE:
        _CACHE['nc'] = build_core_program(cfg)
    res = run_bass_kernel_spmd(_CACHE['nc'], in_maps,
                               core_ids=list(range(cfg.n_cores)))
    return assemble_output(res.results, cfg)


# revision 12
# speedup vs baseline: 1.0846x; 1.0846x over previous
"""Trainium2 Bass kernel for nn_DecoderRoPEBlock (B=4, LQ=1024, LC=512,
E=1024, H=16, FF=4096) running SPMD on 8 NeuronCores.

Sharding: 8 cores = (batch, striped query-tiles); zero collectives.
Stage-1 causal self-attention K/V are recomputed per core from the
original x (causality means the pre-residual x suffices), so each core
produces its 512 output rows independently. Causal striping: each core
owns interleaved 128-token query tiles, so score/exp/AV work shrinks by
the causal factor uniformly across all cores.

v2 optimizations over the baseline:
- Score tiles hold a pair of key-tiles with equal causal offset, so one
  Exp activation covers two key tiles (halves ACT instruction count).
- Softmax denominator reciprocal on DVE (vector.reciprocal) instead of
  Ln+Exp on the Scalar engine; the recip broadcast is read from PSUM by
  the normalizing STT (no SBUF staging copy).
- Cross-attention K/V projections (which depend only on `context`) are
  emitted as PE filler work interleaved into the stage-1 attention loop,
  covering the Tensor-engine bubbles left by the softmax critical path.
- PSUM plan: mm ring (2 banks) + score-pair ring (2x2 banks) + o_pair
  (2 banks) = 8 banks; fc2 reuses all four as 8 accumulators.
- Mask loads shrunk to the single 128-col block each key tile needs.
"""
import sys
sys.path.insert(0, '/opt/trn_rl_repo')
from contextlib import ExitStack

import numpy as np
import ml_dtypes

import concourse.bass as bass
from concourse import bacc
import concourse.tile as tile
import concourse.mybir as mybir

f32 = mybir.dt.float32
bf16 = mybir.dt.bfloat16
f8 = mybir.dt.float8e4
AF = mybir.ActivationFunctionType
ALU = mybir.AluOpType
EPS = 1e-6
P = 128


class Cfg:
    def __init__(self, E, H, LQ, LC, B, FF, n_cores):
        self.E, self.H, self.LQ, self.LC, self.B, self.FF = E, H, LQ, LC, B, FF
        self.HD = E // H
        assert self.HD == 64, "rope layout assumes head dim 64"
        self.n_cores = n_cores
        self.qsplit = n_cores // B
        assert B * self.qsplit == n_cores
        self.Lq = LQ // self.qsplit
        assert self.Lq <= 512
        self.Lk = LQ
        self.Lc = LC
        self.nec = E // P
        self.nkt = self.Lk // P
        self.nct = self.Lc // P
        self.npr = H // 2
        assert self.npr == self.nec, "2 heads per 128-chunk layout"
        self.nft = FF // P
        self.NT = 512


def su_list_sa(c):
    return [min(P * (kt % (c.nkt // 2)), c.Lq - P) for kt in range(c.nkt)]


def _swap32_dma(nc, dst, src, L):
    """dst = src with 32-blocks swapped inside each 64-block (partitions).
    Issued on the gpsimd queue to keep the sync HWDGE queue free for
    weight streaming."""
    nc.gpsimd.dma_start(out=dst[0:32], in_=src[32:64])
    nc.gpsimd.dma_start(out=dst[32:64], in_=src[0:32])
    nc.gpsimd.dma_start(out=dst[64:96], in_=src[96:128])
    nc.gpsimd.dma_start(out=dst[96:128], in_=src[64:96])


def _steer_act_tables(arch):
    """Keep Ln/Exp/Square together in natural_log_exp_and_others so the
    LN rstd pipeline (Ln -> Exp) and softmax Exp never reload tables."""
    from concourse.hw_specs import get_activation_tables
    try:
        tabs = get_activation_tables(arch)
    except Exception:
        return
    target = 'natural_log_exp_and_others'
    if target not in tabs:
        return
    keep = tabs[target]
    for name, s in tabs.items():
        if name == target:
            continue
        if AF.Exp in keep:
            s.discard(AF.Exp)
        if AF.Ln in keep:
            s.discard(AF.Ln)
        if AF.Square in keep:
            s.discard(AF.Square)


def build_core_program(cfg: Cfg):
    c = cfg
    nc = bacc.Bacc()
    _steer_act_tables(nc.m.arch)

    d_xT = nc.declare_dram_parameter("xT", [c.E, c.Lq], f32, isOutput=False)
    d_xT16 = nc.declare_dram_parameter("xT16", [c.E, c.Lk], bf16, isOutput=False)
    d_ctxT16 = nc.declare_dram_parameter("ctxT16", [c.E, c.Lc], bf16, isOutput=False)
    d_mask = nc.declare_dram_parameter("mask16", [c.Lk, P], bf16, isOutput=False)
    WNAMES = ["sa_q", "sa_k", "sa_v", "sa_p", "ca_q", "ca_k", "ca_v", "ca_p"]
    d_w = {n: nc.declare_dram_parameter("w_" + n, [c.E, c.E], bf16, isOutput=False)
           for n in WNAMES}
    d_fc1 = nc.declare_dram_parameter("w_fc1", [c.E, c.FF], bf16, isOutput=False)
    d_fc2 = nc.declare_dram_parameter("w_fc2", [c.FF, c.E], f8, isOutput=False)
    d_cq = nc.declare_dram_parameter("cos_q", [P, c.Lq], bf16, isOutput=False)
    d_sq = nc.declare_dram_parameter("sin_q", [P, c.Lq], bf16, isOutput=False)
    d_ck = nc.declare_dram_parameter("cos_k", [P, c.Lk], bf16, isOutput=False)
    d_sk = nc.declare_dram_parameter("sin_k", [P, c.Lk], bf16, isOutput=False)
    d_cc = nc.declare_dram_parameter("cos_c", [P, c.Lc], bf16, isOutput=False)
    d_sc = nc.declare_dram_parameter("sin_c", [P, c.Lc], bf16, isOutput=False)
    d_g = nc.declare_dram_parameter("gvec", [c.E, 3], f32, isOutput=False)
    d_out = nc.declare_dram_parameter("outT", [c.E, c.Lq], f32, isOutput=True)

    Lq, Lk, Lc, nec, nkt, nct, npr, nft = (
        c.Lq, c.Lk, c.Lc, c.nec, c.nkt, c.nct, c.npr, c.nft)
    VNT = min(c.NT, c.E)
    n_vnt = c.E // VNT
    KNT = min(c.NT, Lk)
    n_knt = Lk // KNT
    NG = 4 if nft % 4 == 0 else 1          # fc1/fc2 stream groups
    FG = c.FF // NG                        # cols per fc1 group
    nftg = nft // NG

    with tile.TileContext(nc) as tc, ExitStack() as ctx:
        # -------------------- pools --------------------
        p_x = ctx.enter_context(tc.tile_pool(name="p_x", bufs=1))
        p_h = ctx.enter_context(tc.tile_pool(name="p_h", bufs=1))
        p_big = ctx.enter_context(tc.tile_pool(name="p_big", bufs=9))
        p_s1k = ctx.enter_context(tc.tile_pool(name="p_s1k", bufs=32))
        p_ex = ctx.enter_context(tc.tile_pool(name="p_ex", bufs=3))
        p_w = ctx.enter_context(tc.tile_pool(name="p_w", bufs=8))
        p_wca = ctx.enter_context(tc.tile_pool(name="p_wca", bufs=8))
        p_ca = ctx.enter_context(tc.tile_pool(name="p_ca", bufs=1))
        p_per = ctx.enter_context(tc.tile_pool(name="p_per", bufs=1))
        p_scr = ctx.enter_context(tc.tile_pool(name="p_scr", bufs=1))
        p_sm = ctx.enter_context(tc.tile_pool(name="p_sm", bufs=1))
        p_t = ctx.enter_context(tc.tile_pool(name="p_t", bufs=3))
        ps_mm = ctx.enter_context(tc.tile_pool(name="ps_mm", bufs=2, space="PSUM"))
        ps_sc = ctx.enter_context(tc.tile_pool(name="ps_sc", bufs=2, space="PSUM"))
        ps_o = ctx.enter_context(tc.tile_pool(name="ps_o", bufs=1, space="PSUM"))

        # -------------------- prologue loads --------------------
        x16 = []
        for e in range(nec):
            t = p_h.tile([P, Lk], bf16, tag=f"h{e}", name=f"h{e}")
            nc.sync.dma_start(out=t[:], in_=d_xT16[e * P:(e + 1) * P, :])
            x16.append(t)
        cq = p_per.tile([P, Lq], bf16, tag="cq", name="cq")
        sq = p_per.tile([P, Lq], bf16, tag="sq", name="sq")
        ck = p_per.tile([P, Lk], bf16, tag="ck", name="ck")
        sk = p_per.tile([P, Lk], bf16, tag="sk", name="sk")
        ccos = p_per.tile([P, Lc], bf16, tag="ccos", name="ccos")
        csin = p_per.tile([P, Lc], bf16, tag="csin", name="csin")
        for t, d in ((cq, d_cq), (sq, d_sq), (ck, d_ck), (sk, d_sk),
                     (ccos, d_cc), (csin, d_sc)):
            nc.sync.dma_start(out=t[:], in_=d[:, :])
        masks = []
        for kt in range(nkt):
            t = p_per.tile([P, P], bf16, tag=f"mask{kt}", name=f"mask{kt}")
            nc.gpsimd.dma_start(out=t[:], in_=d_mask[kt * P:(kt + 1) * P, :])
            masks.append(t)
        ctx16 = []
        for e in range(nec):
            t = p_ca.tile([P, Lc], bf16, tag=f"ctx{e}", name=f"ctx{e}")
            nc.gpsimd.dma_start(out=t[:], in_=d_ctxT16[e * P:(e + 1) * P, :])
            ctx16.append(t)
        gsb = p_per.tile([P, nec, 3], f32, tag="g", name="g")
        for e in range(nec):
            nc.gpsimd.dma_start(out=gsb[:, e, :], in_=d_g[e * P:(e + 1) * P, :])
        ones_col = p_per.tile([P, 1], bf16, tag="ones_col", name="ones_col")
        nc.vector.memset(ones_col[:], 1.0)
        ones_row = p_per.tile([1, P], bf16, tag="ones_row", name="ones_row")
        nc.vector.memset(ones_row[:], 1.0)
        epsb = p_per.tile([1, 1], f32, tag="epsb", name="epsb")
        nc.vector.memset(epsb[:], EPS)

        def load_w(dram, pool=None, tag="wproj"):
            pool = pool or p_w
            tiles = []
            for e in range(nec):
                t = pool.tile([P, c.E], bf16, tag=tag, name=f"{tag}_{e}")
                nc.sync.dma_start(out=t[:], in_=dram[e * P:(e + 1) * P, :])
                tiles.append(t)
            return tiles

        # ca_k / ca_v weights up front; their GEMMs run inside stage 1
        w_cak = load_w(d_w["ca_k"], pool=p_wca, tag="wca")
        w_cav = load_w(d_w["ca_v"], pool=p_wca, tag="wca")

        # xT (f32 residual base) is only needed from the sa_p residual on
        xT = []
        for e in range(nec):
            t = p_x.tile([P, Lq], f32, tag=f"x{e}", name=f"x{e}")
            nc.sync.dma_start(out=t[:], in_=d_xT[e * P:(e + 1) * P, :])
            xT.append(t)

        # ==================== LN ====================
        def layer_norm(src_tiles, L, src_f32=None):
            """LN over E of transposed src [E, L] -> bf16 h tiles (p_h h{e})."""
            n_lt = max(1, L // 512)
            LT = L // n_lt
            # phase A: per-L-subtile stats -> rstd, cc vectors (SBUF)
            rstds, ccvs = [], []
            for lt in range(n_lt):
                sl = slice(lt * LT, (lt + 1) * LT)
                sq_t = []
                for e in range(nec):
                    s = p_scr.tile([P, LT], bf16, tag=f"sq{e}", name=f"sq{e}")
                    nc.vector.tensor_mul(s[:], src_tiles[e][:, sl],
                                         src_tiles[e][:, sl])
                    sq_t.append(s)
                s1 = ps_mm.tile([1, LT], f32, tag="mm", name="s1")
                s2 = ps_mm.tile([1, LT], f32, tag="mm", name="s2")
                for e in range(nec):
                    nc.tensor.matmul(s1[:], ones_col[:], src_tiles[e][:, sl],
                                     start=(e == 0), stop=(e == nec - 1))
                for e in range(nec):
                    nc.tensor.matmul(s2[:], ones_col[:], sq_t[e][:],
                                     start=(e == 0), stop=(e == nec - 1))
                mu = p_sm.tile([1, LT], f32, tag="lnsc", name="mu", bufs=4)
                nc.scalar.mul(mu[:], s1[:], 1.0 / c.E)
                mu2 = p_sm.tile([1, LT], f32, tag="lnsc", name="mu2", bufs=4)
                nc.scalar.square(mu2[:], mu[:])
                s2c = p_sm.tile([1, LT], f32, tag="lnsc", name="s2c", bufs=4)
                nc.scalar.mul(s2c[:], s2[:], 1.0 / c.E)
                var = p_sm.tile([1, LT], f32, tag="lnsc", name="var", bufs=4)
                nc.vector.tensor_sub(var[:], s2c[:], mu2[:])
                lnv = p_sm.tile([1, LT], f32, tag="lnsc", name="lnv", bufs=4)
                nc.scalar.activation(out=lnv[:], in_=var[:], func=AF.Ln,
                                     bias=epsb[:])
                rstd = p_sm.tile([1, LT], bf16, tag="rstd", name="rstd", bufs=2)
                nc.scalar.activation(out=rstd[:], in_=lnv[:], func=AF.Exp,
                                     scale=-0.5)
                ccv = p_sm.tile([1, LT], bf16, tag="ccv", name="ccv", bufs=2)
                nc.vector.tensor_mul(ccv[:], mu[:], rstd[:])
                rstds.append(rstd)
                ccvs.append(ccv)
            # phase B: broadcasts (lt=0 -> mm ring, lt=1 -> sc ring)
            rstd_bs, cc_bs = [], []
            for lt in range(n_lt):
                if lt == 0:
                    rb = ps_mm.tile([P, LT], f32, tag="mm", name="rb")
                    cb = ps_mm.tile([P, LT], f32, tag="mm", name="cb")
                else:
                    rb = ps_sc.tile([P, LT], f32, tag="sc", name="rb1")
                    cb = ps_sc.tile([P, LT], f32, tag="sc", name="cb1")
                nc.tensor.matmul(rb[:], ones_row[:], rstds[lt][:],
                                 start=True, stop=True)
                nc.tensor.matmul(cb[:], ones_row[:], ccvs[lt][:],
                                 start=True, stop=True)
                rstd_bs.append(rb)
                cc_bs.append(cb)
            # phase C: apply; fully read src chunk e before writing h[e]
            hs = [p_h.tile([P, L], bf16, tag=f"h{e}", name=f"hln{e}")
                  for e in range(nec)]
            for e in range(nec):
                src = src_f32[e] if src_f32 is not None else src_tiles[e]
                tmps = []
                for lt in range(n_lt):
                    sl = slice(lt * LT, (lt + 1) * LT)
                    tmp = p_t.tile([P, LT], bf16, tag="lntmp", name="lntmp", bufs=2)
                    nc.vector.tensor_mul(tmp[:], src[:, sl], rstd_bs[lt][:])
                    tmps.append(tmp)
                for lt in range(n_lt):
                    sl = slice(lt * LT, (lt + 1) * LT)
                    nc.vector.tensor_sub(hs[e][:, sl], tmps[lt][:], cc_bs[lt][:])
            return hs

        # ==================== projections ====================
        def project_T(w_tiles, rhs_tiles, L, out_tag, pool):
            """[E_out, L] = sum_e w[e].T @ rhs[e]; returns nec bf16 tiles."""
            outs = []
            for eo in range(nec):
                ps = ps_mm.tile([P, L], f32, tag="mm", name="mm")
                for ei in range(nec):
                    nc.tensor.matmul(ps[:], w_tiles[ei][:, eo * P:(eo + 1) * P],
                                     rhs_tiles[ei][:], start=(ei == 0),
                                     stop=(ei == nec - 1))
                o = pool.tile([P, L], bf16, tag=f"{out_tag}{eo}", name=f"{out_tag}{eo}")
                nc.scalar.copy(o[:], ps[:])
                outs.append(o)
            return outs

        def rope_combine(dst_ap, raw_tile, cos_ap, sin_ap, L):
            swp = p_t.tile([P, L], bf16, tag="ropeswp", name="ropeswp", bufs=2)
            _swap32_dma(nc, swp[:], raw_tile[:], L)
            t1 = p_t.tile([P, L], bf16, tag="ropet1", name="ropet1", bufs=2)
            nc.vector.tensor_mul(t1[:], raw_tile[:], cos_ap)
            t2 = p_t.tile([P, L], bf16, tag="ropet2", name="ropet2", bufs=2)
            nc.vector.tensor_mul(t2[:], swp[:], sin_ap)
            nc.vector.tensor_add(dst_ap, t1[:], t2[:])

        def v_project2(w_tiles, rhs_tiles, n_kt, v_tag, use_vector):
            v_sb = []
            for kt in range(n_kt):
                vt = p_per.tile([P, c.H, 65], bf16, tag=f"{v_tag}{kt}",
                                name=f"{v_tag}{kt}")
                nc.vector.memset(vt[:, :, 64:65], 1.0)
                v_sb.append(vt)
            for kt in range(n_kt):
                for vn in range(n_vnt):
                    ps = ps_mm.tile([P, VNT], f32, tag="mm", name="mm")
                    for ei in range(nec):
                        nc.tensor.matmul(
                            ps[:],
                            rhs_tiles[ei][:, kt * P:(kt + 1) * P],
                            w_tiles[ei][:, vn * VNT:(vn + 1) * VNT],
                            start=(ei == 0), stop=(ei == nec - 1))
                    nh = VNT // 64
                    dst = v_sb[kt][:, vn * nh:(vn + 1) * nh, 0:64]
                    src = ps[:].rearrange("p (nh d) -> p nh d", d=64)
                    if use_vector:
                        nc.vector.tensor_copy(dst, src)
                    else:
                        nc.scalar.copy(dst, src)
            return v_sb

        # ==================== attention ====================
        def attention(q_tiles, k_tiles, v_sb, n_kt, use_mask, su_list,
                      fillers=None, fill_per_pr=0):
            """Key tiles are processed in pairs (kt, kt + n_kt//2) with equal
            causal start column su, so one Exp covers both. Returns Onorm
            tiles (tag qt{pr}). fillers: iterator of thunks emitted between
            pr iterations to keep the PE busy during softmax latency."""
            npair = n_kt // 2
            on_tiles = []
            for pr in range(npr):
                qch = q_tiles[pr]
                kch = k_tiles[pr]
                o_pair = ps_o.tile([65, 2, Lq], f32, tag="o", name="o_pair")
                for hh_i, (hh, pbase) in enumerate(
                        ((2 * pr, 0), (2 * pr + 1, 64))):
                    exs = []
                    for pi in range(npair):
                        kts = (pi, pi + npair)
                        su = su_list[kts[0]]
                        assert su == su_list[kts[1]]
                        s_ps = ps_sc.tile([P, 2, Lq], f32, tag="sc", name="s_ps")
                        for i, kt in enumerate(kts):
                            nc.tensor.matmul(
                                s_ps[:, i, su:],
                                kch[pbase:pbase + 64, kt * P:(kt + 1) * P],
                                qch[pbase:pbase + 64, su:],
                                start=True, stop=True)
                        ex = p_ex.tile([P, 2, Lq], bf16, tag="ex", name="ex")
                        nc.scalar.activation(out=ex[:, :, su:],
                                             in_=s_ps[:, :, su:],
                                             func=AF.Exp, scale=0.125)
                        if use_mask:
                            for i, kt in enumerate(kts):
                                nc.vector.tensor_mul(
                                    ex[:, i, su:su + P],
                                    ex[:, i, su:su + P],
                                    masks[kt][:, :])
                        exs.append((ex, su, kts))
                    # AV chain over all key tiles for this head
                    nmm = 0
                    for (ex, su, kts) in exs:
                        for i, kt in enumerate(kts):
                            nc.tensor.matmul(
                                o_pair[:, hh_i, su:],
                                v_sb[kt][:, hh, :],
                                ex[:, i, su:],
                                start=(nmm == 0), stop=(nmm == n_kt - 1))
                            nmm += 1
                # normalization: recip on DVE, broadcast via PE, STT on DVE
                lnd = p_sm.tile([1, 2, Lq], bf16, tag="rec", name="lnd", bufs=2)
                nc.scalar.activation(out=lnd[:], in_=o_pair[64:65, :, :],
                                     func=AF.Ln)
                rec = p_sm.tile([1, 2, Lq], bf16, tag="rec", name="rec", bufs=2)
                nc.scalar.activation(out=rec[:], in_=lnd[:],
                                     func=AF.Exp, scale=-1.0)
                db_ps = ps_mm.tile([P, Lq], f32, tag="mm", name="db_ps")
                for h in (0, 1):
                    nc.tensor.matmul(db_ps[h * 64:(h + 1) * 64, :],
                                     ones_row[:, 0:64], rec[:, h, :],
                                     start=True, stop=True)
                db = p_t.tile([P, Lq], bf16, tag="db", name="db", bufs=2)
                nc.vector.tensor_copy(db[:], db_ps[:])
                on = p_per.tile([P, Lq], bf16, tag=f"qt{pr}", name=f"on{pr}")
                for h, pbase in ((0, 0), (1, 64)):
                    nc.vector.scalar_tensor_tensor(
                        out=on[pbase:pbase + 64, :],
                        in0=o_pair[0:64, h, :], scalar=1.0,
                        in1=db[h * 64:(h + 1) * 64, :],
                        op0=ALU.bypass, op1=ALU.mult)
                on_tiles.append(on)
                if fillers is not None:
                    for _ in range(fill_per_pr):
                        try:
                            next(fillers)()
                        except StopIteration:
                            break
            return on_tiles

        def proj_residual(w_tiles, src_tiles, g_idx):
            for e in range(nec):
                ps = ps_mm.tile([P, Lq], f32, tag="mm", name="mm")
                for ei in range(nec):
                    nc.tensor.matmul(ps[:], w_tiles[ei][:, e * P:(e + 1) * P],
                                     src_tiles[ei][:], start=(ei == 0),
                                     stop=(ei == nec - 1))
                nc.vector.scalar_tensor_tensor(
                    out=xT[e][:], in0=ps[:], scalar=gsb[:, e, g_idx:g_idx + 1],
                    in1=xT[e][:], op0=ALU.mult, op1=ALU.add)

        # ========== cross-attn K/V filler work (runs inside stage 1) =====
        kt2 = [None] * nec
        v2 = []
        for kt in range(nct):
            vt = p_ca.tile([P, c.H, 65], bf16, tag=f"v2_{kt}", name=f"v2_{kt}")
            nc.vector.memset(vt[:, :, 64:65], 1.0)
            v2.append(vt)

        def ca_k_chunk(eo):
            def thunk():
                ps = ps_mm.tile([P, Lc], f32, tag="mm", name="cak")
                for ei in range(nec):
                    nc.tensor.matmul(ps[:],
                                     w_cak[ei][:, eo * P:(eo + 1) * P],
                                     ctx16[ei][:], start=(ei == 0),
                                     stop=(ei == nec - 1))
                raw = p_t.tile([P, Lc], bf16, tag="k2raw", name="k2raw", bufs=2)
                nc.vector.tensor_copy(raw[:], ps[:])
                full = p_ca.tile([P, Lc], bf16, tag=f"kt2_{eo}", name=f"kt2_{eo}")
                rope_combine(full[:], raw, ccos[:], csin[:], Lc)
                kt2[eo] = full
            return thunk

        def ca_v_chunk(kt, vn):
            def thunk():
                ps = ps_mm.tile([P, VNT], f32, tag="mm", name="cav")
                for ei in range(nec):
                    nc.tensor.matmul(
                        ps[:],
                        ctx16[ei][:, kt * P:(kt + 1) * P],
                        w_cav[ei][:, vn * VNT:(vn + 1) * VNT],
                        start=(ei == 0), stop=(ei == nec - 1))
                nh = VNT // 64
                nc.vector.tensor_copy(
                    v2[kt][:, vn * nh:(vn + 1) * nh, 0:64],
                    ps[:].rearrange("p (nh d) -> p nh d", d=64))
            return thunk

        ca_fillers = iter(
            [ca_k_chunk(eo) for eo in range(nec)] +
            [ca_v_chunk(kt, vn) for kt in range(nct) for vn in range(n_vnt)])

        # ==================== STAGE 1: causal self-attention ============
        h1 = layer_norm(x16, Lk)
        hq = [t[:, 0:Lq] for t in h1]

        # 4 cross-attn K chunks fill the PE while LN1-apply runs on DVE
        for _ in range(4):
            next(ca_fillers)()

        w = load_w(d_w["sa_q"])
        q_raw = project_T(w, hq, Lq, "scrB", p_scr)
        qt1 = []
        for pr in range(npr):
            q = p_per.tile([P, Lq], bf16, tag=f"qt{pr}", name=f"qt{pr}")
            rope_combine(q[:], q_raw[pr], cq[:], sq[:], Lq)
            qt1.append(q)
        w = load_w(d_w["sa_k"])
        kt1 = []
        k_raw_nt = []
        for nt in range(n_knt):
            sl = slice(nt * KNT, (nt + 1) * KNT)
            tag = "sq" if nt == 0 else "scrB"
            kr = project_T(w, [t[:, sl] for t in h1], KNT, tag, p_scr)
            k_raw_nt.append(kr)
        for e in range(nec):
            full = p_big.tile([P, Lk], bf16, tag="big", name="big")
            for nt in range(n_knt):
                sl = slice(nt * KNT, (nt + 1) * KNT)
                rope_combine(full[:, sl], k_raw_nt[nt][e],
                             ck[:, sl], sk[:, sl], KNT)
            kt1.append(full)
        w = load_w(d_w["sa_v"])
        v1 = v_project2(w, h1, nkt, "v", use_vector=False)
        su_sa = su_list_sa(c)
        on1 = attention(qt1, kt1, v1, nkt, True, su_sa,
                        fillers=ca_fillers, fill_per_pr=2)
        # drain any remaining filler work
        for th in ca_fillers:
            th()
        w = load_w(d_w["sa_p"])
        proj_residual(w, on1, 0)

        # ==================== STAGE 2: cross-attention ==================
        x16_2 = []
        for e in range(nec):
            t = p_scr.tile([P, Lq], bf16, tag=f"scrB{e}", name=f"scrB{e}")
            nc.scalar.copy(t[:], xT[e][:])
            x16_2.append(t)
        h2 = layer_norm(x16_2, Lq, src_f32=xT)
        w = load_w(d_w["ca_q"])
        q_raw = project_T(w, h2, Lq, "scrB", p_scr)
        qt2 = []
        for pr in range(npr):
            q = p_per.tile([P, Lq], bf16, tag=f"qt{pr}", name=f"qt{pr}")
            rope_combine(q[:], q_raw[pr], cq[:], sq[:], Lq)
            qt2.append(q)
        # prefetch fc1 group 0 weights during stage-2 attention
        wf0 = []
        for e in range(nec):
            t = p_big.tile([P, FG], bf16, tag="big", name="big")
            nc.sync.dma_start(out=t[:], in_=d_fc1[e * P:(e + 1) * P, 0:FG])
            wf0.append(t)
        on2 = attention(qt2, kt2, v2, nct, False, [0] * nct)
        w = load_w(d_w["ca_p"])
        proj_residual(w, on2, 1)

        # ==================== STAGE 3: MLP ==============================
        x16_3 = []
        for e in range(nec):
            t = p_scr.tile([P, Lq], bf16, tag=f"scrB{e}", name=f"scrB{e}")
            nc.scalar.copy(t[:], xT[e][:])
            x16_3.append(t)
        h3 = layer_norm(x16_3, Lq, src_f32=xT)
        a_tiles = []
        for grp in range(NG):
            if grp == 0:
                wf = wf0
            else:
                wf = []
                for e in range(nec):
                    t = p_big.tile([P, FG], bf16, tag="big", name="big")
                    nc.sync.dma_start(
                        out=t[:], in_=d_fc1[e * P:(e + 1) * P,
                                            grp * FG:(grp + 1) * FG])
                    wf.append(t)
            for ft in range(nftg):
                ps = ps_mm.tile([P, Lq], f32, tag="mm", name="mm")
                for ei in range(nec):
                    nc.tensor.matmul(ps[:], wf[ei][:, ft * P:(ft + 1) * P],
                                     h3[ei][:], start=(ei == 0),
                                     stop=(ei == nec - 1))
                a = p_s1k.tile([P, Lq], f8, tag="s1k", name="s1k")
                nc.scalar.activation(out=a[:], in_=ps[:],
                                     func=AF.Gelu_apprx_tanh)
                a_tiles.append(a)
        # fc2: all 8 psum banks as accumulators, stream fc2 weights
        accs = []
        for i in range(2):
            big = ps_sc.tile([P, 2, Lq], f32, tag="sc", name=f"fc2sc{i}")
            accs.append(big[:, 0, :])
            accs.append(big[:, 1, :])
        op = ps_o.tile([P, 2, Lq], f32, tag="o", name="fc2o")
        accs.append(op[:, 0, :])
        accs.append(op[:, 1, :])
        for i in range(2):
            accs.append(ps_mm.tile([P, Lq], f32, tag="mm", name=f"fc2mm{i}"))
        assert len(accs) >= nec
        for fi in range(nft):
            wt = p_w.tile([P, c.E], f8, tag="wproj", name="wproj")
            nc.sync.dma_start(out=wt[:], in_=d_fc2[fi * P:(fi + 1) * P, :])
            for e in range(nec):
                nc.tensor.matmul(accs[e][:], wt[:, e * P:(e + 1) * P],
                                 a_tiles[fi][:], start=(fi == 0),
                                 stop=(fi == nft - 1))
        for e in range(nec):
            nc.vector.scalar_tensor_tensor(
                out=xT[e][:], in0=accs[e][:], scalar=gsb[:, e, 2:3],
                in1=xT[e][:], op0=ALU.mult, op1=ALU.add)

        # ==================== output ====================
        for e in range(nec):
            nc.sync.dma_start(out=d_out[e * P:(e + 1) * P, :], in_=xT[e][:])

    nc.finalize()
    return nc


# ======================================================================
# Host-side preparation
# ======================================================================
def rope_tables(positions, HD, dtype=np.float32):
    inv_freq = 1.0 / (10000.0 ** (np.arange(0, HD, 2, dtype=np.float64) / HD))
    ang = positions[None, :].astype(np.float64) * inv_freq[:, None]
    cos, sin = np.cos(ang), np.sin(ang)
    c64 = np.concatenate([cos, cos], 0)
    s64 = np.concatenate([-sin, sin], 0)
    return (np.concatenate([c64, c64], 0).astype(dtype),
            np.concatenate([s64, s64], 0).astype(dtype))


def rope_perm(E, HD):
    H = E // HD
    perm = np.zeros(E, dtype=np.int64)
    for h in range(H):
        base = h * HD
        perm[base:base + 32] = base + np.arange(0, HD, 2)
        perm[base + 32:base + HD] = base + np.arange(1, HD, 2)
    return perm


def to_bf(a):
    return np.asarray(a, dtype=np.float32).astype(ml_dtypes.bfloat16)


def host_prep(inputs, cfg: Cfg):
    c = cfg
    E, HD = c.E, c.HD
    perm = rope_perm(E, HD)

    def ln_fold(w, nw, do_perm):
        weff = np.asarray(w, np.float64)
        if nw is not None:
            weff = weff * np.asarray(nw, np.float64)[None, :]
        if do_perm:
            weff = weff[perm, :]
        return weff.T

    x = np.asarray(inputs['x'], np.float32)
    ctxv = np.asarray(inputs['context'], np.float32)
    am = np.asarray(inputs['attn_mask'])
    n1w, n2w, n3w = (np.asarray(inputs[k], np.float32).reshape(-1)
                     for k in ('n1_w', 'n2_w', 'n3_w'))
    for nb in ('n1_b', 'n2_b', 'n3_b', 'sa_qb', 'sa_kb', 'sa_vb', 'sa_pb',
               'ca_qb', 'ca_kb', 'ca_vb', 'ca_pb', 'fc1_b', 'fc2_b'):
        assert not np.any(np.asarray(inputs[nb])), f"nonzero bias {nb}"

    shared = {
        'w_sa_q': to_bf(ln_fold(inputs['sa_qw'], n1w, True)),
        'w_sa_k': to_bf(ln_fold(inputs['sa_kw'], n1w, True)),
        'w_sa_v': to_bf(ln_fold(inputs['sa_vw'], n1w, False)),
        'w_sa_p': to_bf(np.asarray(inputs['sa_pw'], np.float64).T),
        'w_ca_q': to_bf(ln_fold(inputs['ca_qw'], n2w, True)),
        'w_ca_k': to_bf(ln_fold(inputs['ca_kw'], None, True)),
        'w_ca_v': to_bf(np.asarray(inputs['ca_vw'], np.float64).T),
        'w_ca_p': to_bf(np.asarray(inputs['ca_pw'], np.float64).T),
        'w_fc1': to_bf(ln_fold(inputs['fc1_w'], n3w, False)),
        'w_fc2': np.ascontiguousarray(
            (np.asarray(inputs['fc2_w'], np.float64).T * 32.0)
            .astype(np.float32).astype(ml_dtypes.float8_e4m3fn)),
        'gvec': np.ascontiguousarray(np.stack(
            [np.asarray(inputs['g_msa'], np.float32).reshape(-1),
             np.asarray(inputs['g_ca'], np.float32).reshape(-1),
             np.asarray(inputs['g_mlp'], np.float32).reshape(-1) / 32.0], 1)),
    }
    mask_T = (~am).astype(np.float32).T       # [k, q] multiplicative
    cc_np, sc_np = rope_tables(np.arange(c.Lc), HD)
    shared['cos_c'] = to_bf(cc_np)
    shared['sin_c'] = to_bf(sc_np)
    su_sa = su_list_sa(c)

    in_maps = []
    for core in range(c.n_cores):
        b = core // c.qsplit
        qh = core % c.qsplit
        perm_tok = _core_token_perm(c, qh)
        own = perm_tok[:c.Lq]
        cq_np, sq_np = rope_tables(own.astype(np.float64), HD)
        ck_np, sk_np = rope_tables(perm_tok.astype(np.float64), HD)
        xb_T = x[b].T                          # [E, Lk]
        # per-kt [P, P] mask blocks at the causal start column su
        mask_full = to_bf(mask_T[perm_tok][:, own])     # [Lk, Lq]
        mask_blk = np.zeros((c.Lk, P), ml_dtypes.bfloat16)
        for kt in range(c.nkt):
            su = su_sa[kt]
            mask_blk[kt * P:(kt + 1) * P, :] = \
                mask_full[kt * P:(kt + 1) * P, su:su + P]
        m = dict(shared)
        m.update({
            'xT': np.ascontiguousarray(xb_T[:, own]),
            'xT16': np.ascontiguousarray(to_bf(xb_T[:, perm_tok])),
            'ctxT16': np.ascontiguousarray(to_bf(ctxv[b].T)),
            'mask16': np.ascontiguousarray(mask_blk),
            'cos_q': to_bf(cq_np), 'sin_q': to_bf(sq_np),
            'cos_k': to_bf(ck_np), 'sin_k': to_bf(sk_np),
        })
        in_maps.append(m)
    return in_maps


def _core_token_perm(c, qh):
    """Own (striped) 128-token tiles first, then the other range's tiles."""
    ntile = c.Lk // 128
    if c.qsplit == 1:
        order = list(range(ntile))
    else:
        own_t = list(range(qh, ntile, c.qsplit))
        oth_t = [t for t in range(ntile) if t not in own_t]
        order = own_t + oth_t
    return np.concatenate(
        [np.arange(t * 128, (t + 1) * 128) for t in order])


def assemble_output(results, cfg: Cfg):
    c = cfg
    out = np.zeros((c.B, c.LQ, c.E), np.float32)
    for core in range(c.n_cores):
        b = core // c.qsplit
        qh = core % c.qsplit
        own = _core_token_perm(c, qh)[:c.Lq]
        out[b, own, :] = np.asarray(results[core]['outT']).T
    return out


_CFG = Cfg(E=1024, H=16, LQ=1024, LC=512, B=4, FF=4096, n_cores=8)
_CACHE = {}


def kernel(**inputs):
    from concourse.bass_utils import run_bass_kernel_spmd
    cfg = _CFG
    in_maps = host_prep(inputs, cfg)
    if 'nc' not in _CACHE:
        _CACHE['nc'] = build_core_program(cfg)
    res = run_bass_kernel_spmd(_CACHE['nc'], in_maps,
                               core_ids=list(range(cfg.n_cores)))
    return assemble_output(res.results, cfg)
